# revision 5
# baseline (speedup 1.0000x reference)
"""Multi-head attention (B=4, S=1024, D=1024, H=16) on 8 Trainium2 NeuronCores.

Sharding: core c handles batch b=c//2 and query-half q=c%2 (512 query rows).
Each core computes K/V projections for its batch (duplicated within the
batch pair -> no collectives), Q projection for its query rows, attention
for all 16 heads over its 512 query rows, and the output projection for its
512 rows.  Host concatenates the 8 [512, 1024] results.

v2 changes vs baseline (310.6us -> 146.6us in the CoreSim cost model):
  - all DRAM operands are host-pre-transposed (contraction-major) and cast
    to bf16, so every DMA is contiguous (no 2x small-element penalty) and
    half-size; PSUM accumulation stays fp32
  - DMAs split across two queues: SP (sync) streams activations + output,
    Pool (gpsimd) streams weights + constants; small constants batched
    into single [128, 8] loads; vnat "ones" columns via memset
  - V projection runs as k-outer waves over 6 concurrent PSUM chains so
    PE starts as soon as the first (xv, wv) tile pair lands
  - per head-pair, the next pair's K/Q projections are issued between the
    last scores and last ctx matmuls, hiding the ACT exp pipeline drain;
    K(si0)/Q/K(si1) order hides the PSUM->SBUF bias drains
  - output projection emits natural [q, dout] layout with interleaved
    PSUM chains (proj_ps + retired ctx_ps banks) and contiguous stores
"""

import sys

for _p in ("/opt/trn_rl_repo", "/opt/pypackages"):
    if _p not in sys.path:
        sys.path.append(_p)

import numpy as np

B = 4
S = 1024
D = 1024
H = 16
HD = 64
SQ = 512          # query rows per core
KT = D // 128     # 8 contraction tiles
SKT = S // 128    # 8 key tiles
QT = SQ // 128    # 4 query tiles per core
NCORES = 8

_COMPILED = None


def _build():
    import concourse.bass as bass
    import concourse.mybir as mybir
    from concourse import bacc
    from concourse.bass import ts
    from concourse.tile import TileContext

    f32 = mybir.dt.float32
    bf16 = mybir.dt.bfloat16
    i32 = mybir.dt.int32
    EXP = mybir.ActivationFunctionType.Exp

    nc = bacc.Bacc("TRN2", target_bir_lowering=False, debug=False,
                   num_devices=NCORES)

    # host-pre-transposed operands (contraction-major), bf16
    xq_d = nc.dram_tensor("xqT", [D, SQ], bf16, kind="ExternalInput")
    xk_d = nc.dram_tensor("xkT", [D, S], bf16, kind="ExternalInput")
    xv_d = nc.dram_tensor("xvT", [D, S], bf16, kind="ExternalInput")
    mask_d = nc.dram_tensor("mask", [S], i32, kind="ExternalInput")
    wq_d = nc.dram_tensor("WqT", [D, D], bf16, kind="ExternalInput")
    wk_d = nc.dram_tensor("WkT", [D, D], bf16, kind="ExternalInput")
    wv_d = nc.dram_tensor("WvT", [D, D], bf16, kind="ExternalInput")
    wo_d = nc.dram_tensor("WoT", [D, D], bf16, kind="ExternalInput")
    bq_d = nc.dram_tensor("bq", [D], f32, kind="ExternalInput")
    bk_d = nc.dram_tensor("bk", [D], f32, kind="ExternalInput")
    bv_d = nc.dram_tensor("bv", [D], f32, kind="ExternalInput")
    bo_d = nc.dram_tensor("bo", [D], f32, kind="ExternalInput")
    out_d = nc.dram_tensor("out", [SQ, D], f32, kind="ExternalOutput")

    with TileContext(nc) as tc:
        from contextlib import ExitStack
        with ExitStack() as stack:
            const = stack.enter_context(tc.tile_pool(name="const", bufs=1))
            vnat_p = stack.enter_context(tc.tile_pool(name="vnat", bufs=1))
            ctx_p = stack.enter_context(tc.tile_pool(name="ctxT", bufs=1))

            # ---- weight tiles (Pool queue), activation tiles (SP queue) ----
            # wv/xv first on both queues: the first V matmul only needs the
            # k=0 pair, so PE starts ~1.7us in
            wv_p = stack.enter_context(tc.tile_pool(name="wv", bufs=1))
            xv_p = stack.enter_context(tc.tile_pool(name="xv", bufs=1))
            wv_t, xv_t = [], []
            for k in range(KT):
                t = wv_p.tile([128, D], bf16, tag=f"wv{k}")
                nc.gpsimd.dma_start(t[:], wv_d[ts(k, 128), :])
                wv_t.append(t)
                t = xv_p.tile([128, S], bf16, tag=f"xv{k}")
                nc.sync.dma_start(t[:], xv_d[ts(k, 128), :])
                xv_t.append(t)

            # ---- constants (Pool queue) -----------------------------------
            # mask/bq/bk batched: [128, 8] where col t = elems t*128..t*128+128
            mi8 = const.tile([128, SKT], i32, tag="mi8")
            nc.gpsimd.dma_start(mi8[:], mask_d[:].rearrange(
                "(a b) -> b a", a=SKT))
            mf8 = const.tile([128, SKT], f32, tag="mf8")
            nc.vector.tensor_copy(mf8[:], mi8[:])
            mb8 = const.tile([128, SKT], f32, tag="mb8")
            # (mask - 1) * 1e9  ->  0 for keep, -1e9 for masked
            nc.vector.tensor_scalar(mb8[:], mf8[:], 1e9, -1e9,
                                    mybir.AluOpType.mult,
                                    mybir.AluOpType.add)
            bq8 = const.tile([128, KT], f32, tag="bq8")
            nc.gpsimd.dma_start(bq8[:], bq_d[:].rearrange(
                "(a b) -> b a", a=KT))
            bk8 = const.tile([128, KT], f32, tag="bk8")
            nc.gpsimd.dma_start(bk8[:], bk_d[:].rearrange(
                "(a b) -> b a", a=KT))

            # vnat "ones" columns: memsets queued on Pool before the
            # remaining weight DMAs so they complete long before the first
            # ctx matmul
            vnat = [vnat_p.tile([128, H * 65], bf16, tag=f"v{m}",
                                name=f"vnat{m}")
                    for m in range(SKT)]
            for m in range(SKT):
                vv = vnat[m][:].rearrange("p (h x) -> p h x", x=65)
                nc.gpsimd.memset(vv[:, :, 64:65], 1.0)

            bv_bc = const.tile([128, D], f32, tag="bvbc")
            nc.gpsimd.dma_start(
                bv_bc[:],
                bass.AP(tensor=bv_d, offset=0, ap=[[0, 128], [1, D]]))

            wqk_p = stack.enter_context(tc.tile_pool(name="wqk", bufs=1))
            xk_p = stack.enter_context(tc.tile_pool(name="xk", bufs=1))
            xq_p = stack.enter_context(tc.tile_pool(name="xq", bufs=1))
            wk_t, xk_t = [], []
            for k in range(KT):
                t = wqk_p.tile([128, D], bf16, tag=f"wk{k}", name=f"wkt{k}")
                nc.gpsimd.dma_start(t[:], wk_d[ts(k, 128), :])
                wk_t.append(t)
                t = xk_p.tile([128, S], bf16, tag=f"xk{k}")
                nc.sync.dma_start(t[:], xk_d[ts(k, 128), :])
                xk_t.append(t)
            wq_t, xq_t = [], []
            for k in range(KT):
                t = wqk_p.tile([128, D], bf16, tag=f"wq{k}", name=f"wqt{k}")
                nc.gpsimd.dma_start(t[:], wq_d[ts(k, 128), :])
                wq_t.append(t)
                t = xq_p.tile([128, SQ], bf16, tag=f"xq{k}")
                nc.sync.dma_start(t[:], xq_d[ts(k, 128), :])
                xq_t.append(t)

            bo_bc = const.tile([128, D], f32, tag="bobc")
            nc.gpsimd.dma_start(
                bo_bc[:],
                bass.AP(tensor=bo_d, offset=0, ap=[[0, 128], [1, D]]))
            wo_p = stack.enter_context(tc.tile_pool(name="wo", bufs=1))
            wo_t = []
            for k in range(KT):
                t = wo_p.tile([128, D], bf16, tag=f"wo{k}", name=f"wot{k}")
                nc.gpsimd.dma_start(t[:], wo_d[ts(k, 128), :])
                wo_t.append(t)

            # ---- V projection: vnat[m] = [128 keys, 16 heads x (64+1)] ----
            # k-outer waves: 6 concurrent PSUM chains (m=0..5) consume each
            # (xv[k], wv[k]) tile pair as it lands; m=6,7 run as regular
            # rotating groups on proj_ps, which stays open for the whole
            # kernel so the K-projection never waits on a pool transition.
            ctxT = [ctx_p.tile([128, SQ], bf16, tag=f"c{k}", name=f"ctxT{k}")
                    for k in range(KT)]
            proj_ps = stack.enter_context(
                tc.tile_pool(name="proj_ps", bufs=2, space="PSUM"))

            def vdrain(m, n, ps):
                vv = vnat[m][:].rearrange("p (h x) -> p h x", x=65)
                nc.vector.tensor_add(
                    vv[:, 8 * n:8 * n + 8, 0:64],
                    ps[:].rearrange("p (h x) -> p h x", x=64),
                    bv_bc[:, ts(n, 512)].rearrange("p (h x) -> p h x", x=64))

            NW = 6
            with tc.tile_pool(name="vwave_ps", bufs=NW, space="PSUM") as vw_ps:
                for n in range(2):
                    pss = [vw_ps.tile([128, 512], f32, tag="vw",
                                      name=f"vw{n}_{m}")
                           for m in range(NW)]
                    for k in range(KT):
                        for m in range(NW):
                            nc.tensor.matmul(
                                pss[m][:], xv_t[k][:, ts(m, 128)],
                                wv_t[k][:, ts(n, 512)],
                                start=(k == 0), stop=(k == KT - 1))
                    for m in range(NW):
                        vdrain(m, n, pss[m])
                    for m in range(NW, SKT):
                        ps = proj_ps.tile([128, 512], f32, tag="pp")
                        for k in range(KT):
                            nc.tensor.matmul(
                                ps[:], xv_t[k][:, ts(m, 128)],
                                wv_t[k][:, ts(n, 512)],
                                start=(k == 0), stop=(k == KT - 1))
                        vdrain(m, n, ps)

            # ---- per head-pair: K/Q projection + attention -----------------
            with tc.tile_pool(name="scores_ps", bufs=2, space="PSUM") \
                    as scores_ps, \
                 tc.tile_pool(name="ctx_ps", bufs=1, space="PSUM") \
                    as ctx_ps, \
                 tc.tile_pool(name="qkT", bufs=2) as qkT_p, \
                 tc.tile_pool(name="e", bufs=2) as e_p, \
                 tc.tile_pool(name="nrm", bufs=2) as nrm_p, \
                 tc.tile_pool(name="outN", bufs=3) as out_p:

                def emit_proj(hp):
                    # K(si=0), Q, K(si=1): later matmul groups keep PE busy
                    # while DVE drains the earlier PSUM groups.
                    khT = qkT_p.tile([128, S], bf16, tag="khT")
                    ps = proj_ps.tile([128, 512], f32, tag="pp")
                    for k in range(KT):
                        nc.tensor.matmul(
                            ps[:], wk_t[k][:, ts(hp, 128)],
                            xk_t[k][:, ts(0, 512)],
                            start=(k == 0), stop=(k == KT - 1))
                    nc.vector.tensor_scalar_add(
                        khT[:, ts(0, 512)], ps[:], bk8[:, hp:hp + 1])
                    qhT = qkT_p.tile([128, SQ], bf16, tag="qhT")
                    ps = proj_ps.tile([128, 512], f32, tag="pp")
                    for k in range(KT):
                        nc.tensor.matmul(
                            ps[:], wq_t[k][:, ts(hp, 128)], xq_t[k][:],
                            start=(k == 0), stop=(k == KT - 1))
                    nc.vector.tensor_scalar_add(qhT[:], ps[:],
                                                bq8[:, hp:hp + 1])
                    ps = proj_ps.tile([128, 512], f32, tag="pp")
                    for k in range(KT):
                        nc.tensor.matmul(
                            ps[:], wk_t[k][:, ts(hp, 128)],
                            xk_t[k][:, ts(1, 512)],
                            start=(k == 0), stop=(k == KT - 1))
                    nc.vector.tensor_scalar_add(
                        khT[:, ts(1, 512)], ps[:], bk8[:, hp:hp + 1])
                    return khT, qhT

                def outproj_mms(pss, pair, ks):
                    for k in ks:
                        for i, (qt, half) in enumerate(pair):
                            nc.tensor.matmul(
                                pss[i][:], ctxT[k][:, ts(qt, 128)],
                                wo_t[k][:, ts(half, 512)],
                                start=(k == 0), stop=(k == KT - 1))

                def outproj_alloc(pair, pool, tags):
                    return [pool.tile([128, 512], f32, tag=tag,
                                      name=f"op{qt}_{half}")
                            for (qt, half), tag in zip(pair, tags)]

                def outproj_drain(pss, pair):
                    for i, (qt, half) in enumerate(pair):
                        ot = out_p.tile([128, 512], f32, tag="o")
                        nc.vector.tensor_add(ot[:], pss[i][:],
                                             bo_bc[:, ts(half, 512)])
                        nc.sync.dma_start(
                            out_d[ts(qt, 128), ts(half, 512)], ot[:])

                khT, qhT = emit_proj(0)
                for hp in range(H // 2):
                    # attention for heads a=2*hp (partitions 0:64) and
                    # b=2*hp+1 (partitions 64:128); ctx(t) is issued after
                    # scores(t+1) so PE has work while ACT computes exp(t),
                    # and the next head-pair's projections run between
                    # scores(7) and ctx(7) so the ACT pipeline drain is
                    # fully hidden
                    a, b = 2 * hp, 2 * hp + 1
                    psCa = ctx_ps.tile([128, 512], f32, tag="ca")
                    psCb = ctx_ps.tile([128, 512], f32, tag="cb")
                    eTs = [None] * SKT

                    def scores_t(t, khT=khT, qhT=qhT):
                        psS = scores_ps.tile([128, 1024], f32, tag="s")
                        nc.tensor.matmul(
                            psS[:, 0:512], khT[0:64, ts(t, 128)],
                            qhT[0:64, :], start=True, stop=True)
                        nc.tensor.matmul(
                            psS[:, 512:1024], khT[64:128, ts(t, 128)],
                            qhT[64:128, :], start=True, stop=True,
                            tile_position=(64, 0))
                        eT = e_p.tile([128, 1024], bf16, tag="e")
                        nc.scalar.activation(eT[:], psS[:], EXP,
                                             bias=mb8[:, t:t + 1],
                                             scale=1.0 / np.sqrt(HD))
                        eTs[t] = eT

                    def ctx_t(t, psCa=psCa, psCb=psCb, a=a, b=b):
                        st, sp = (t == 0), (t == SKT - 1)
                        eT = eTs[t]
                        nc.tensor.matmul(
                            psCa[0:65, :], vnat[t][:, ts(a, 65)],
                            eT[:, 0:512], start=st, stop=sp)
                        nc.tensor.matmul(
                            psCb[0:65, :], vnat[t][:, ts(b, 65)],
                            eT[:, 512:1024], start=st, stop=sp)

                    last = hp == H // 2 - 1
                    scores_t(0)
                    for t in range(1, SKT):
                        scores_t(t)
                        ctx_t(t - 1)
                    if not last:
                        khT, qhT = emit_proj(hp + 1)
                        ctx_t(SKT - 1)
                    else:
                        # fill the ACT-pipeline drain with the first output
                        # projection pair's k<7 matmuls (k=7 needs this
                        # head-pair's normalization, so it comes after)
                        pair1 = ((0, 0), (0, 1))
                        pss1 = outproj_alloc(pair1, proj_ps, ("pp", "pp"))
                        outproj_mms(pss1, pair1, range(3))
                        ctx_t(SKT - 1)

                    for half, psC in ((0, psCa), (1, psCb)):
                        rec = nrm_p.tile([1, 512], f32, tag=f"r{half}")
                        nc.vector.reciprocal(rec[:], psC[64:65, :])
                        bc = nrm_p.tile([64, 512], f32, tag=f"b{half}")
                        nc.gpsimd.partition_broadcast(bc[:], rec[:])
                        nc.vector.tensor_mul(
                            ctxT[hp][64 * half:64 * half + 64, :],
                            psC[0:64, :], bc[:])
                    if last:
                        outproj_mms(pss1, pair1, range(3, KT - 1))

                # ---- output projection (natural [q, dout] layout) ----------
                # chains alternate between proj_ps and the now-dead ctx_ps
                # banks so each pair's matmuls cover the previous pair's
                # PSUM drain latency
                outproj_mms(pss1, pair1, [KT - 1])
                groups = [(qt, half) for qt in range(QT) for half in range(2)]
                rest = [g for g in groups if g not in pair1]
                prev_pss, prev_pair = pss1, pair1
                for i in range(0, len(rest), 2):
                    pair = (rest[i], rest[i + 1])
                    pool, tags = ((ctx_ps, ("ca", "cb")) if i % 4 == 0
                                  else (proj_ps, ("pp", "pp")))
                    pss = outproj_alloc(pair, pool, tags)
                    outproj_mms(pss, pair, range(KT))
                    outproj_drain(prev_pss, prev_pair)
                    prev_pss, prev_pair = pss, pair
                outproj_drain(prev_pss, prev_pair)

    nc.compile()
    return nc


def _get_compiled():
    global _COMPILED
    if _COMPILED is None:
        _COMPILED = _build()
    return _COMPILED


def _bf16(a):
    import ml_dtypes
    return np.ascontiguousarray(np.asarray(a, np.float32).astype(
        ml_dtypes.bfloat16))


def _common_map(inputs):
    common = {
        "WqT": _bf16(np.asarray(inputs["Wq"], np.float32).T),
        "WkT": _bf16(np.asarray(inputs["Wk"], np.float32).T),
        "WvT": _bf16(np.asarray(inputs["Wv"], np.float32).T),
        "WoT": _bf16(np.asarray(inputs["Wo"], np.float32).T),
    }
    for n in ("bq", "bk", "bv", "bo"):
        common[n] = np.ascontiguousarray(np.asarray(inputs[n], np.float32))
    return common


def _core_in_map(c, q, k, v, mask, inputs, _cache={}):
    # keep a reference to q as the cache key so its id can't be recycled
    if _cache.get("qref") is not q:
        _cache.clear()
        _cache["qref"] = q
        _cache["common"] = _common_map(inputs)
        _cache["kT"] = [_bf16(k[b].T) for b in range(B)]
        _cache["vT"] = [_bf16(v[b].T) for b in range(B)]
    bidx, qh = c // 2, c % 2
    return {
        "xqT": _bf16(q[bidx, qh * SQ:(qh + 1) * SQ, :].T),
        "xkT": _cache["kT"][bidx],
        "xvT": _cache["vT"][bidx],
        "mask": np.ascontiguousarray(mask[bidx, 0]),
        **_cache["common"],
    }


def _expected_shard(c, expected):
    bidx, qh = c // 2, c % 2
    return expected[bidx, qh * SQ:(qh + 1) * SQ, :]


def kernel(q, k, v, mask, Wq, bq, Wk, bk, Wv, bv, Wo, bo, **_ignored):
    from concourse.bass_utils import run_bass_kernel_spmd

    nc = _get_compiled()
    q = np.asarray(q, dtype=np.float32)
    k = np.asarray(k, dtype=np.float32)
    v = np.asarray(v, dtype=np.float32)
    mask = np.asarray(mask, dtype=np.int32)
    inputs = {"Wq": Wq, "Wk": Wk, "Wv": Wv, "Wo": Wo,
              "bq": bq, "bk": bk, "bv": bv, "bo": bo}
    in_maps = [_core_in_map(c, q, k, v, mask, inputs) for c in range(NCORES)]
    res = run_bass_kernel_spmd(nc, in_maps, core_ids=list(range(NCORES)))
    out = np.empty((B, S, D), np.float32)
    for c in range(NCORES):
        bidx, qh = c // 2, c % 2
        out[bidx, qh * SQ:(qh + 1) * SQ, :] = res.results[c]["out"]
    return out


# revision 6
# speedup vs baseline: 1.0176x; 1.0176x over previous
"""Multi-head attention (B=4, S=1024, D=1024, H=16) on 8 Trainium2 NeuronCores.

Sharding: core c handles batch b=c//2 and query-half q=c%2 (512 query rows).
Each core computes K/V projections for its batch (duplicated within the
batch pair -> no collectives), Q projection for its query rows, attention
for all 16 heads over its 512 query rows, and the output projection for its
512 rows.  Host concatenates the 8 [512, 1024] results.

v2 changes vs baseline (310.6us -> 146.6us in the CoreSim cost model):
  - all DRAM operands are host-pre-transposed (contraction-major) and cast
    to bf16, so every DMA is contiguous (no 2x small-element penalty) and
    half-size; PSUM accumulation stays fp32
  - DMAs split across two queues: SP (sync) streams activations + output,
    Pool (gpsimd) streams weights + constants; small constants batched
    into single [128, 8] loads; vnat "ones" columns via memset
  - V projection runs as k-outer waves over 6 concurrent PSUM chains so
    PE starts as soon as the first (xv, wv) tile pair lands
  - per head-pair, the next pair's K/Q projections are issued between the
    last scores and last ctx matmuls, hiding the ACT exp pipeline drain;
    K(si0)/Q/K(si1) order hides the PSUM->SBUF bias drains
  - output projection emits natural [q, dout] layout with interleaved
    PSUM chains (proj_ps + retired ctx_ps banks) and contiguous stores
"""

import sys

for _p in ("/opt/trn_rl_repo", "/opt/pypackages"):
    if _p not in sys.path:
        sys.path.append(_p)

import numpy as np

B = 4
S = 1024
D = 1024
H = 16
HD = 64
SQ = 512          # query rows per core
KT = D // 128     # 8 contraction tiles
SKT = S // 128    # 8 key tiles
QT = SQ // 128    # 4 query tiles per core
NCORES = 8

_COMPILED = None


def _build():
    import concourse.bass as bass
    import concourse.mybir as mybir
    from concourse import bacc
    from concourse.bass import ts
    from concourse.tile import TileContext

    f32 = mybir.dt.float32
    bf16 = mybir.dt.bfloat16
    i32 = mybir.dt.int32
    EXP = mybir.ActivationFunctionType.Exp

    nc = bacc.Bacc("TRN2", target_bir_lowering=False, debug=False,
                   num_devices=NCORES)

    # host-pre-transposed operands (contraction-major), bf16
    xq_d = nc.dram_tensor("xqT", [D, SQ], bf16, kind="ExternalInput")
    xk_d = nc.dram_tensor("xkT", [D, S], bf16, kind="ExternalInput")
    xv_d = nc.dram_tensor("xvT", [D, S], bf16, kind="ExternalInput")
    mask_d = nc.dram_tensor("mask", [S], i32, kind="ExternalInput")
    wq_d = nc.dram_tensor("WqT", [D, D], bf16, kind="ExternalInput")
    wk_d = nc.dram_tensor("WkT", [D, D], bf16, kind="ExternalInput")
    wv_d = nc.dram_tensor("WvT", [D, D], bf16, kind="ExternalInput")
    wo_d = nc.dram_tensor("WoT", [D, D], bf16, kind="ExternalInput")
    bq_d = nc.dram_tensor("bq", [D], f32, kind="ExternalInput")
    bk_d = nc.dram_tensor("bk", [D], f32, kind="ExternalInput")
    bv_d = nc.dram_tensor("bv", [D], f32, kind="ExternalInput")
    bo_d = nc.dram_tensor("bo", [D], f32, kind="ExternalInput")
    out_d = nc.dram_tensor("out", [SQ, D], f32, kind="ExternalOutput")

    with TileContext(nc) as tc:
        from contextlib import ExitStack
        with ExitStack() as stack:
            const = stack.enter_context(tc.tile_pool(name="const", bufs=1))
            vnat_p = stack.enter_context(tc.tile_pool(name="vnat", bufs=1))
            ctx_p = stack.enter_context(tc.tile_pool(name="ctxT", bufs=1))

            # ---- weight tiles (Pool queue), activation tiles (SP queue) ----
            # wv/xv first on both queues: the first V matmul only needs the
            # k=0 pair, so PE starts ~1.7us in
            wv_p = stack.enter_context(tc.tile_pool(name="wv", bufs=1))
            xv_p = stack.enter_context(tc.tile_pool(name="xv", bufs=1))
            wv_t, xv_t = [], []
            for k in range(KT):
                t = wv_p.tile([128, D], bf16, tag=f"wv{k}")
                nc.gpsimd.dma_start(t[:], wv_d[ts(k, 128), :])
                wv_t.append(t)
                t = xv_p.tile([128, S], bf16, tag=f"xv{k}")
                nc.sync.dma_start(t[:], xv_d[ts(k, 128), :])
                xv_t.append(t)

            # ---- constants (Pool queue) -----------------------------------
            # mask/bq/bk batched: [128, 8] where col t = elems t*128..t*128+128
            mi8 = const.tile([128, SKT], i32, tag="mi8")
            nc.gpsimd.dma_start(mi8[:], mask_d[:].rearrange(
                "(a b) -> b a", a=SKT))
            mf8 = const.tile([128, SKT], f32, tag="mf8")
            nc.vector.tensor_copy(mf8[:], mi8[:])
            mb8 = const.tile([128, SKT], f32, tag="mb8")
            # (mask - 1) * 1e9  ->  0 for keep, -1e9 for masked
            nc.vector.tensor_scalar(mb8[:], mf8[:], 1e9, -1e9,
                                    mybir.AluOpType.mult,
                                    mybir.AluOpType.add)
            bq8 = const.tile([128, KT], f32, tag="bq8")
            nc.gpsimd.dma_start(bq8[:], bq_d[:].rearrange(
                "(a b) -> b a", a=KT))
            bk8 = const.tile([128, KT], f32, tag="bk8")
            nc.gpsimd.dma_start(bk8[:], bk_d[:].rearrange(
                "(a b) -> b a", a=KT))

            # vnat "ones" columns: memsets queued on Pool before the
            # remaining weight DMAs so they complete long before the first
            # ctx matmul
            vnat = [vnat_p.tile([128, H * 65], bf16, tag=f"v{m}",
                                name=f"vnat{m}")
                    for m in range(SKT)]
            for m in range(SKT):
                vv = vnat[m][:].rearrange("p (h x) -> p h x", x=65)
                nc.gpsimd.memset(vv[:, :, 64:65], 1.0)

            bv_bc = const.tile([128, D], f32, tag="bvbc")
            nc.gpsimd.dma_start(
                bv_bc[:],
                bass.AP(tensor=bv_d, offset=0, ap=[[0, 128], [1, D]]))

            wqk_p = stack.enter_context(tc.tile_pool(name="wqk", bufs=1))
            xk_p = stack.enter_context(tc.tile_pool(name="xk", bufs=1))
            xq_p = stack.enter_context(tc.tile_pool(name="xq", bufs=1))
            wk_t, xk_t = [], []
            for k in range(KT):
                t = wqk_p.tile([128, D], bf16, tag=f"wk{k}", name=f"wkt{k}")
                nc.gpsimd.dma_start(t[:], wk_d[ts(k, 128), :])
                wk_t.append(t)
                t = xk_p.tile([128, S], bf16, tag=f"xk{k}")
                nc.sync.dma_start(t[:], xk_d[ts(k, 128), :])
                xk_t.append(t)
            wq_t, xq_t = [], []
            for k in range(KT):
                t = wqk_p.tile([128, D], bf16, tag=f"wq{k}", name=f"wqt{k}")
                nc.gpsimd.dma_start(t[:], wq_d[ts(k, 128), :])
                wq_t.append(t)
                t = xq_p.tile([128, SQ], bf16, tag=f"xq{k}")
                nc.sync.dma_start(t[:], xq_d[ts(k, 128), :])
                xq_t.append(t)

            bo_bc = const.tile([128, D], f32, tag="bobc")
            nc.gpsimd.dma_start(
                bo_bc[:],
                bass.AP(tensor=bo_d, offset=0, ap=[[0, 128], [1, D]]))
            wo_p = stack.enter_context(tc.tile_pool(name="wo", bufs=1))
            wo_t = []
            for k in range(KT):
                t = wo_p.tile([128, D], bf16, tag=f"wo{k}", name=f"wot{k}")
                nc.gpsimd.dma_start(t[:], wo_d[ts(k, 128), :])
                wo_t.append(t)

            # ---- V projection: vnat[m] = [128 keys, 16 heads x (64+1)] ----
            # k-outer waves: 6 concurrent PSUM chains (m=0..5) consume each
            # (xv[k], wv[k]) tile pair as it lands; m=6,7 run as regular
            # rotating groups on proj_ps, which stays open for the whole
            # kernel so the K-projection never waits on a pool transition.
            ctxT = [ctx_p.tile([128, SQ], bf16, tag=f"c{k}", name=f"ctxT{k}")
                    for k in range(KT)]
            proj_ps = stack.enter_context(
                tc.tile_pool(name="proj_ps", bufs=2, space="PSUM"))

            def vdrain(m, n, ps):
                vv = vnat[m][:].rearrange("p (h x) -> p h x", x=65)
                nc.vector.tensor_add(
                    vv[:, 8 * n:8 * n + 8, 0:64],
                    ps[:].rearrange("p (h x) -> p h x", x=64),
                    bv_bc[:, ts(n, 512)].rearrange("p (h x) -> p h x", x=64))

            NW = 6
            with tc.tile_pool(name="vwave_ps", bufs=NW, space="PSUM") as vw_ps:
                for n in range(2):
                    pss = [vw_ps.tile([128, 512], f32, tag="vw",
                                      name=f"vw{n}_{m}")
                           for m in range(NW)]
                    for k in range(KT):
                        for m in range(NW):
                            nc.tensor.matmul(
                                pss[m][:], xv_t[k][:, ts(m, 128)],
                                wv_t[k][:, ts(n, 512)],
                                start=(k == 0), stop=(k == KT - 1))
                    for m in range(NW):
                        vdrain(m, n, pss[m])
                    for m in range(NW, SKT):
                        ps = proj_ps.tile([128, 512], f32, tag="pp")
                        for k in range(KT):
                            nc.tensor.matmul(
                                ps[:], xv_t[k][:, ts(m, 128)],
                                wv_t[k][:, ts(n, 512)],
                                start=(k == 0), stop=(k == KT - 1))
                        vdrain(m, n, ps)

            # ---- per head-pair: K/Q projection + attention -----------------
            with tc.tile_pool(name="scores_ps", bufs=2, space="PSUM") \
                    as scores_ps, \
                 tc.tile_pool(name="ctx_ps", bufs=1, space="PSUM") \
                    as ctx_ps, \
                 tc.tile_pool(name="qkT", bufs=2) as qkT_p, \
                 tc.tile_pool(name="e", bufs=2) as e_p, \
                 tc.tile_pool(name="nrm", bufs=2) as nrm_p, \
                 tc.tile_pool(name="outN", bufs=3) as out_p:

                def emit_proj(hp):
                    # K(si=0), Q, K(si=1): later matmul groups keep PE busy
                    # while DVE drains the earlier PSUM groups.
                    khT = qkT_p.tile([128, S], bf16, tag="khT")
                    ps = proj_ps.tile([128, 512], f32, tag="pp")
                    for k in range(KT):
                        nc.tensor.matmul(
                            ps[:], wk_t[k][:, ts(hp, 128)],
                            xk_t[k][:, ts(0, 512)],
                            start=(k == 0), stop=(k == KT - 1))
                    nc.vector.tensor_scalar_add(
                        khT[:, ts(0, 512)], ps[:], bk8[:, hp:hp + 1])
                    qhT = qkT_p.tile([128, SQ], bf16, tag="qhT")
                    ps = proj_ps.tile([128, 512], f32, tag="pp")
                    for k in range(KT):
                        nc.tensor.matmul(
                            ps[:], wq_t[k][:, ts(hp, 128)], xq_t[k][:],
                            start=(k == 0), stop=(k == KT - 1))
                    nc.vector.tensor_scalar_add(qhT[:], ps[:],
                                                bq8[:, hp:hp + 1])
                    ps = proj_ps.tile([128, 512], f32, tag="pp")
                    for k in range(KT):
                        nc.tensor.matmul(
                            ps[:], wk_t[k][:, ts(hp, 128)],
                            xk_t[k][:, ts(1, 512)],
                            start=(k == 0), stop=(k == KT - 1))
                    nc.vector.tensor_scalar_add(
                        khT[:, ts(1, 512)], ps[:], bk8[:, hp:hp + 1])
                    return khT, qhT

                def outproj_mms(pss, pair, ks):
                    for k in ks:
                        for i, (qt, half) in enumerate(pair):
                            nc.tensor.matmul(
                                pss[i], ctxT[k][:, ts(qt, 128)],
                                wo_t[k][:, ts(half, 512)],
                                start=(k == 0), stop=(k == KT - 1))

                def outproj_alloc(pair, pool, tags, width=512):
                    # chains are [128, 512]; when borrowing the retired
                    # [128, 1024] scores_ps tiles, use their first half
                    return [pool.tile([128, width], f32, tag=tag,
                                      name=f"op{qt}_{half}")[:, 0:512]
                            for (qt, half), tag in zip(pair, tags)]

                def outproj_drain(pss, pair):
                    for i, (qt, half) in enumerate(pair):
                        ot = out_p.tile([128, 512], f32, tag="o")
                        nc.vector.tensor_add(ot[:], pss[i],
                                             bo_bc[:, ts(half, 512)])
                        nc.sync.dma_start(
                            out_d[ts(qt, 128), ts(half, 512)], ot[:])

                khT, qhT = emit_proj(0)
                for hp in range(H // 2):
                    # attention for heads a=2*hp (partitions 0:64) and
                    # b=2*hp+1 (partitions 64:128); ctx(t) is issued after
                    # scores(t+1) so PE has work while ACT computes exp(t),
                    # and the next head-pair's projections run between
                    # scores(7) and ctx(7) so the ACT pipeline drain is
                    # fully hidden
                    a, b = 2 * hp, 2 * hp + 1
                    psCa = ctx_ps.tile([128, 512], f32, tag="ca")
                    psCb = ctx_ps.tile([128, 512], f32, tag="cb")
                    eTs = [None] * SKT

                    def scores_t(t, khT=khT, qhT=qhT):
                        psS = scores_ps.tile([128, 1024], f32, tag="s")
                        nc.tensor.matmul(
                            psS[:, 0:512], khT[0:64, ts(t, 128)],
                            qhT[0:64, :], start=True, stop=True)
                        nc.tensor.matmul(
                            psS[:, 512:1024], khT[64:128, ts(t, 128)],
                            qhT[64:128, :], start=True, stop=True,
                            tile_position=(64, 0))
                        eT = e_p.tile([128, 1024], bf16, tag="e")
                        nc.scalar.activation(eT[:], psS[:], EXP,
                                             bias=mb8[:, t:t + 1],
                                             scale=1.0 / np.sqrt(HD))
                        eTs[t] = eT

                    def ctx_t(t, psCa=psCa, psCb=psCb, a=a, b=b):
                        st, sp = (t == 0), (t == SKT - 1)
                        eT = eTs[t]
                        nc.tensor.matmul(
                            psCa[0:65, :], vnat[t][:, ts(a, 65)],
                            eT[:, 0:512], start=st, stop=sp)
                        nc.tensor.matmul(
                            psCb[0:65, :], vnat[t][:, ts(b, 65)],
                            eT[:, 512:1024], start=st, stop=sp)

                    last = hp == H // 2 - 1
                    scores_t(0)
                    for t in range(1, SKT):
                        scores_t(t)
                        ctx_t(t - 1)
                    if not last:
                        khT, qhT = emit_proj(hp + 1)
                        ctx_t(SKT - 1)
                    else:
                        # fill the ACT-pipeline drain and this head-pair's
                        # normalization latency with the first two output
                        # projection pairs' k<7 matmuls (k=7 needs the
                        # normalized ctxT[7], so it comes after)
                        pair1 = ((0, 0), (0, 1))
                        pss1 = outproj_alloc(pair1, proj_ps, ("pp", "pp"))
                        outproj_mms(pss1, pair1, range(KT - 1))
                        ctx_t(SKT - 1)
                        pair2 = ((1, 0), (1, 1))
                        pss2 = outproj_alloc(pair2, scores_ps, ("s", "s"),
                                             width=1024)
                        outproj_mms(pss2, pair2, range(KT - 1))

                    for half, psC in ((0, psCa), (1, psCb)):
                        rec = nrm_p.tile([1, 512], f32, tag=f"r{half}")
                        nc.vector.reciprocal(rec[:], psC[64:65, :])
                        bc = nrm_p.tile([64, 512], f32, tag=f"b{half}")
                        nc.gpsimd.partition_broadcast(bc[:], rec[:])
                        nc.vector.tensor_mul(
                            ctxT[hp][64 * half:64 * half + 64, :],
                            psC[0:64, :], bc[:])

                # ---- output projection (natural [q, dout] layout) ----------
                # four chains in flight (proj_ps, retired scores_ps and
                # ctx_ps banks) so every pair's matmuls cover the previous
                # pair's PSUM drain latency and the hp=7 norm
                outproj_mms(pss1, pair1, [KT - 1])
                outproj_mms(pss2, pair2, [KT - 1])
                pair3 = ((2, 0), (2, 1))
                pss3 = outproj_alloc(pair3, ctx_ps, ("ca", "cb"))
                outproj_mms(pss3, pair3, range(KT))
                outproj_drain(pss1, pair1)
                pair4 = ((3, 0), (3, 1))
                pss4 = outproj_alloc(pair4, proj_ps, ("pp", "pp"))
                outproj_mms(pss4, pair4, range(KT))
                outproj_drain(pss2, pair2)
                outproj_drain(pss3, pair3)
                outproj_drain(pss4, pair4)

    nc.compile()
    return nc


def _get_compiled():
    global _COMPILED
    if _COMPILED is None:
        _COMPILED = _build()
    return _COMPILED


def _bf16(a):
    import ml_dtypes
    return np.ascontiguousarray(np.asarray(a, np.float32).astype(
        ml_dtypes.bfloat16))


def _common_map(inputs):
    common = {
        "WqT": _bf16(np.asarray(inputs["Wq"], np.float32).T),
        "WkT": _bf16(np.asarray(inputs["Wk"], np.float32).T),
        "WvT": _bf16(np.asarray(inputs["Wv"], np.float32).T),
        "WoT": _bf16(np.asarray(inputs["Wo"], np.float32).T),
    }
    for n in ("bq", "bk", "bv", "bo"):
        common[n] = np.ascontiguousarray(np.asarray(inputs[n], np.float32))
    return common


def _core_in_map(c, q, k, v, mask, inputs, _cache={}):
    # keep a reference to q as the cache key so its id can't be recycled
    if _cache.get("qref") is not q:
        _cache.clear()
        _cache["qref"] = q
        _cache["common"] = _common_map(inputs)
        _cache["kT"] = [_bf16(k[b].T) for b in range(B)]
        _cache["vT"] = [_bf16(v[b].T) for b in range(B)]
    bidx, qh = c // 2, c % 2
    return {
        "xqT": _bf16(q[bidx, qh * SQ:(qh + 1) * SQ, :].T),
        "xkT": _cache["kT"][bidx],
        "xvT": _cache["vT"][bidx],
        "mask": np.ascontiguousarray(mask[bidx, 0]),
        **_cache["common"],
    }


def _expected_shard(c, expected):
    bidx, qh = c // 2, c % 2
    return expected[bidx, qh * SQ:(qh + 1) * SQ, :]


def kernel(q, k, v, mask, Wq, bq, Wk, bk, Wv, bv, Wo, bo, **_ignored):
    from concourse.bass_utils import run_bass_kernel_spmd

    nc = _get_compiled()
    q = np.asarray(q, dtype=np.float32)
    k = np.asarray(k, dtype=np.float32)
    v = np.asarray(v, dtype=np.float32)
    mask = np.asarray(mask, dtype=np.int32)
    inputs = {"Wq": Wq, "Wk": Wk, "Wv": Wv, "Wo": Wo,
              "bq": bq, "bk": bk, "bv": bv, "bo": bo}
    in_maps = [_core_in_map(c, q, k, v, mask, inputs) for c in range(NCORES)]
    res = run_bass_kernel_spmd(nc, in_maps, core_ids=list(range(NCORES)))
    out = np.empty((B, S, D), np.float32)
    for c in range(NCORES):
        bidx, qh = c // 2, c % 2
        out[bidx, qh * SQ:(qh + 1) * SQ, :] = res.results[c]["out"]
    return out
